# revision 18
# baseline (speedup 1.0000x reference)
"""Trainium2 Bass kernel for nn_CrossAttentionModel (8 NeuronCores).

Design (v4, pair-shared HBM):
  * Cores (2p, 2p+1) share an HBM allocation (addr_space='Shared' DRAM).
    Pair p owns samples [32p, 32p+32); core 2p+j owns d-half j of the
    20480-long contraction.  Each core streams its 80 k-tiles (acts for
    the pair's 32 samples + its W1/W2 half, f16, 7.9 MB) and accumulates
    partial encodings ps[(c,s), e] on-chip.
  * The cross-core reduce is a pair-local 32 KB HBM round-trip: each
    core writes its partial grid (regrouped into two per-half [64, 128]
    blocks) into its slot of the shared tensor, bumps the partner's
    semaphore with a data-less remote_sem_update_broadcast (delta-tpb=1,
    same die), waits for the partner's bump, reads both slots' block for
    its own 16 samples and adds them.  No fabric data transfer, no
    collective on the critical path.
  * A fire-and-forget AllReduce keeps the runtime's gang launch.
  * Attention phase (per-core, 16 samples) exploits the replicated-vis
    structure: the "hi" halves of both attention maps are rank-1, so the
    3x3-mixing stage reduces to three K<=3 matmul groups plus a
    tanh-scale trick; residuals fold into the final PSUM via identity
    matmuls.
"""
import sys
sys.path.insert(0, "/opt/trn_rl_repo")

import numpy as np
import concourse.bass as bass
import concourse.mybir as mybir
import concourse.tile as tile
from concourse.tile_rust import add_dep_helper
from concourse import bacc
from concourse.bass_utils import run_bass_kernel_spmd

F32 = mybir.dt.float32
F16 = mybir.dt.float16

# ---- problem constants (hardcoded; kernel.py must be self-contained) ----
B, C, H, W = 128, 3, 512, 640
D = 20480            # (H//4) * (W//4)
DE = 128             # encoder dim
DA = 32              # attention dim
NC_ = 8              # cores
DL = D // 2          # 10240 d-rows per core (d split across the pair)
NT = DL // 128       # 80 k-tiles per core
SP = 32              # samples per pair
SL = 16              # samples per core (post pair-exchange)
SK = SL * DE         # 2048 = (sample, enc-feat) free columns
NQ = 4               # quads of 4 samples
NCHUNK = 8           # input DMA chunks (10 k-tiles each)
DEBUG = False
KT_PER_CHUNK = NT // NCHUNK
PKW = 96 + 32 + 128 + 128     # per-k-tile packed cols: aT | vT | w1T | w2T

ACT = mybir.ActivationFunctionType


def _np_dt(dt):
    return mybir.dt.np(dt)


def build_bass():
    nc = bacc.Bacc("TRN2", target_bir_lowering=False, debug=False,
                   num_devices=NC_)

    # ---- per-core DRAM parameters ----
    pk = nc.declare_dram_parameter("pk", [128, NT * PKW], F16, isOutput=False)
    bdA = nc.declare_dram_parameter("bdA", [3, 3], F16, isOutput=False)
    rsA = nc.declare_dram_parameter("rsA", [3, 1], F16, isOutput=False)
    csAv = nc.declare_dram_parameter("csAv", [3, 1], F16, isOutput=False)
    alph = nc.declare_dram_parameter("alph", [128, 1], F32, isOutput=False)
    wcaT = nc.declare_dram_parameter("wcaT", [2 * DE, DA], F16, isOutput=False)
    wcvT = nc.declare_dram_parameter("wcvT", [2 * DE, DA], F16, isOutput=False)
    waT = nc.declare_dram_parameter("waT", [3, DA], F16, isOutput=False)
    rsWv = nc.declare_dram_parameter("rsWv", [1, DA], F16, isOutput=False)
    whaT = nc.declare_dram_parameter("whaT", [DA, 3], F16, isOutput=False)
    whvT = nc.declare_dram_parameter("whvT", [DA, 3], F16, isOutput=False)
    i3 = nc.declare_dram_parameter("i3", [3, 3], F16, isOutput=False)
    on13 = nc.declare_dram_parameter("on13", [1, 3], F16, isOutput=False)
    out = nc.declare_dram_parameter("out", [6, SK], F32, isOutput=True)
    if DEBUG:
        dbg_rd = nc.declare_dram_parameter("dbg_rd", [128, 64], F16,
                                           isOutput=True)
        dbg_av = nc.declare_dram_parameter("dbg_av", [64, 128], F16,
                                           isOutput=True)

    # Pair-shared exchange buffer: slot j (rows [128j, 128j+128)) is written
    # by the pair core with j = pid & 1.  Within a slot, block h (rows
    # [64h, 64h+64)) holds the writer's partials for pair-half h samples in
    # (c*16+s | 48+s, e) layout.
    sh = nc.dram_tensor("sh", [256, 128], F16, kind="Internal",
                        addr_space="Shared")

    # Gang-launch collective; nothing waits on its result.
    bar_in = nc.dram_tensor("bar_in", [128, 128], F32)
    bar_out = nc.dram_tensor("bar_out", [128, 128], F32)

    xsem = nc.alloc_semaphore("pair_sem")
    lsem = nc.alloc_semaphore("pair_local_sem")
    # scheduler-sim stand-in for xsem (incremented locally so the single-core
    # scheduling sim can pass the gate; the wait is rewritten post-schedule).
    fksem = nc.alloc_semaphore("fake_pair_sem")

    with tile.TileContext(nc) as tc:
        cc = nc.gpsimd.collective_compute(
            "AllReduce", mybir.AluOpType.add,
            replica_groups=[list(range(NC_))],
            ins=[bar_in[:]], outs=[bar_out[:]],
        )
        with (
            tc.tile_pool(name="consts", bufs=1) as cpool,
            tc.tile_pool(name="sb", bufs=1) as sb,
        ):
            # ---------- persistent SBUF tiles ----------
            pk_t = [cpool.tile([128, KT_PER_CHUNK * PKW], F16,
                               name=f"pk{cix}", tag=f"pk{cix}")
                    for cix in range(NCHUNK)]
            ava_sb = sb.tile([96, 128], F16, name="ava_sb", tag="ava_sb")
            avv_sb = sb.tile([32, 128], F16, name="avv_sb", tag="avv_sb")
            # per-source readbacks, already in attention layout [c|v, (s,k)]
            ba0 = sb.tile([3, SK], F16, name="ba0", tag="ba0")
            ba1 = sb.tile([3, SK], F16, name="ba1", tag="ba1")
            bv0 = sb.tile([1, SK], F16, name="bv0", tag="bv0")
            bv1 = sb.tile([1, SK], F16, name="bv1", tag="bv1")

            # ---------- partner-notify descriptor (prepare-only) ----------
            # Data-carrying 16B-line broadcast: remote_sem_update (no data)
            # and sub-16B payloads crash the SWDGE ucode; [128, 8] f16 works.
            ping_s = sb.tile([128, 8], F16, name="ping_s", tag="ping_s")
            ping_d = sb.tile([128, 8], F16, name="ping_d", tag="ping_d")
            nc.gpsimd.remote_dma_broadcast(
                ping_d[:], ping_s[:],
                remote_sem=xsem, local_sem=lsem,
                rdests=[None, (0, 1), None, None, None, None, None, None])

            # ---------- per-core slot selection ----------
            pid = nc.sync.partition_id()
            jbit = nc.sync.scalar_reg_alu(mybir.AluOpType.bitwise_and, pid, 1)
            jz = nc.sync.scalar_reg_alu(mybir.AluOpType.is_equal, jbit, 0)
            jnz = nc.sync.scalar_reg_alu(mybir.AluOpType.is_equal, jbit, 1)

            # ---------- input chunk loads ----------
            for cix in range(NCHUNK):
                c0 = cix * KT_PER_CHUNK * PKW
                eng = nc.sync if cix % 2 == 0 else nc.scalar
                eng.dma_start(
                    pk_t[cix][:], pk[:, c0:c0 + KT_PER_CHUNK * PKW])

            # ---------- const loads (scalar engine HWDGE) ----------
            bdA_t = cpool.tile([3, 3], F16)
            nc.scalar.dma_start(bdA_t[:], bdA[:])
            rsA_t = cpool.tile([3, 1], F16)
            nc.scalar.dma_start(rsA_t[:], rsA[:])
            csAv_t = cpool.tile([3, 1], F16)
            nc.scalar.dma_start(csAv_t[:], csAv[:])
            alph_t = cpool.tile([128, 1], F32)
            nc.scalar.dma_start(alph_t[:], alph[:])
            wca_lo = cpool.tile([DE, DA], F16)
            nc.scalar.dma_start(wca_lo[:], wcaT[0:DE, :])
            wca_hi = cpool.tile([DE, DA], F16)
            nc.scalar.dma_start(wca_hi[:], wcaT[DE:2 * DE, :])
            wcv_lo = cpool.tile([DE, DA], F16)
            nc.scalar.dma_start(wcv_lo[:], wcvT[0:DE, :])
            wcv_hi = cpool.tile([DE, DA], F16)
            nc.scalar.dma_start(wcv_hi[:], wcvT[DE:2 * DE, :])
            waT_t = cpool.tile([3, DA], F16)
            nc.scalar.dma_start(waT_t[:], waT[:])
            rsWv_t = cpool.tile([1, DA], F16)
            nc.scalar.dma_start(rsWv_t[:], rsWv[:])
            wha_t = cpool.tile([DA, 3], F16)
            nc.scalar.dma_start(wha_t[:], whaT[:])
            whv_t = cpool.tile([DA, 3], F16)
            nc.scalar.dma_start(whv_t[:], whvT[:])
            i3_t = cpool.tile([3, 3], F16)
            nc.scalar.dma_start(i3_t[:], i3[:])
            on13_t = cpool.tile([1, 3], F16)
            nc.scalar.dma_start(on13_t[:], on13[:])

            # ---------- phase 1: encoder (80 k-tiles, full pair batch) ----
            with tc.tile_pool(name="enc_ps", bufs=1, space="PSUM") as eps:
                ps_a = eps.tile([96, 128], F32, name="ps_a")
                ps_v = eps.tile([32, 128], F32, name="ps_v")
                for t in range(NT):
                    cix, tloc = divmod(t, KT_PER_CHUNK)
                    o = tloc * PKW
                    src = pk_t[cix]
                    first, last = t == 0, t == NT - 1
                    nc.tensor.matmul(ps_a[:], src[:, o:o + 96],
                                     src[:, o + 128:o + 256],
                                     start=first, stop=last)
                    nc.tensor.matmul(ps_v[:], src[:, o + 96:o + 128],
                                     src[:, o + 256:o + 384],
                                     start=first, stop=last)
                # evict partial grids (f32 -> f16)
                nc.vector.tensor_copy(ava_sb[:], ps_a[:])
                nc.vector.tensor_copy(avv_sb[:], ps_v[:])

            # ---------- pair exchange over shared HBM ----------
            # write my full grid into slot j, regrouped into per-half blocks
            wr = []
            for slot, cond in ((0, jz), (1, jnz)):
                for h in (0, 1):
                    r0 = 128 * slot + 64 * h
                    for c in range(3):
                        wr.append(nc.sync.dma_start(
                            sh[r0 + 16 * c:r0 + 16 * c + 16, :],
                            ava_sb[32 * c + 16 * h:32 * c + 16 * h + 16, :],
                            cond=cond))
                    wr.append(nc.sync.dma_start(
                        sh[r0 + 48:r0 + 64, :],
                        avv_sb[16 * h:16 * h + 16, :],
                        cond=cond))

            # fire the partner notify once the writes have landed
            nc.vector.memset(ping_s[:], 1.0)
            trig = nc.gpsimd.trigger_dma(
                count=None, signals_writable=[ping_s[:]])
            for w in wr:
                add_dep_helper(trig.ins, w.ins, True,
                               "notify after sh writes land")
            # keep the gang-launch collective off the Pool critical path --
            # the Pool engine blocks on CC completion, which would otherwise
            # gate the exchange trigger behind an ~20us all-core barrier.
            add_dep_helper(cc.ins, trig.ins, False,
                           "dummy AllReduce after exchange trigger")
            nc.gpsimd.sem_inc(fksem, 2)

            # wait for the partner's writes, then read both slots' block for
            # my half (h = j) -- directly regrouped by the DMA into the
            # attention layout [c, (s,k)] / [v, (s,k)].
            fkw = nc.sync.wait_ge(fksem, 2)
            for w in wr:
                add_dep_helper(fkw.ins, w.ins, True,
                               "wait placed after own sh writes")
            rd_dmas = []
            for slot, cond in ((0, jz), (1, jnz)):
                own = 192 * slot                  # 128*slot + 64*slot
                oth = 128 - 64 * slot             # 128*(1-slot) + 64*slot
                for base, dsta, dstv in ((own, ba0, bv0), (oth, ba1, bv1)):
                    rd_dmas.append(nc.sync.dma_start(
                        dsta[:],
                        sh[base:base + 48, :].rearrange(
                            "(c s) k -> c (s k)", s=SL), cond=cond))
                    rd_dmas.append(nc.sync.dma_start(
                        dstv[:], sh[base + 48:base + 64, :], cond=cond))
            for d in rd_dmas:
                add_dep_helper(d.ins, fkw.ins, True,
                               "partner reads gate on pair sem wait")

            av_a = sb.tile([3, SK], F16, name="av_a", tag="av_a")
            av_v = sb.tile([1, SK], F16, name="av_v", tag="av_v")
            nc.vector.tensor_add(av_a[:], ba0[:], ba1[:])
            nc.vector.tensor_add(av_v[:], bv0[:], bv1[:])
            if DEBUG:
                nc.sync.dma_start(dbg_av[:, 0:SK // 2],
                                  av_a[0:1, :].rearrange("o (a b) -> (o a) b",
                                                         a=1))

            # ---------- phase 2: attention (16 samples, on-chip) ----------
            bd_a = sb.tile([3, SK], F16, name="bd_a", tag="bd_a")
            bd_ca = sb.tile([1, SK], F16, name="bd_ca", tag="bd_ca")
            bd_wv = sb.tile([1, SK], F16, name="bd_wv", tag="bd_wv")
            att = {
                (br, half): sb.tile([DE, SK], F16, name=f"att_{br}_{half}",
                                    tag=f"att_{br}_{half}")
                for br in ("a", "v") for half in ("lo", "hi")
            }
            ht_a = sb.tile([DA, SK], F16, name="ht_a", tag="ht_a")
            ht_v = sb.tile([DA, SK], F16, name="ht_v", tag="ht_v")

            with (
                tc.tile_pool(name="bd_ps", bufs=1, space="PSUM") as bps,
                tc.tile_pool(name="att_ps", bufs=2, space="PSUM") as aps,
                tc.tile_pool(name="h_ps", bufs=1, space="PSUM") as hps,
                tc.tile_pool(name="o_ps", bufs=1, space="PSUM") as ops_,
            ):
                # ---- bd stage: aud' rows, csum_a row, w_v row ----
                for q in range(NQ):
                    ck = slice(q * 512, (q + 1) * 512)
                    pa = bps.tile([3, 512], F32, tag="pbd_a")
                    nc.tensor.matmul(pa[:], bdA_t[:], av_a[:, ck],
                                     start=True, stop=True)
                    nc.scalar.copy(bd_a[:, ck], pa[:])
                    pc = bps.tile([1, 512], F32, tag="pbd_c")
                    nc.tensor.matmul(pc[:], rsA_t[:], av_a[:, ck],
                                     start=True, stop=True)
                    nc.scalar.copy(bd_ca[:, ck], pc[:])
                    pw = bps.tile([1, 512], F32, tag="pbd_w")
                    nc.tensor.matmul(pw[:], csAv_t[:], av_a[:, ck],
                                     start=True, stop=True)
                    nc.scalar.copy(bd_wv[:, ck], pw[:])

                # ---- attention maps ----
                aspec = [("a", "lo", av_a, bd_a), ("a", "hi", av_v, bd_ca),
                         ("v", "lo", bd_wv, av_v), ("v", "hi", av_v, av_v)]
                for q in range(NQ):
                    for br, half, lhs_t, rhs_t in aspec:
                        pt = aps.tile([DE, 512], F32, tag="att_ps")
                        for j in range(4):
                            s = q * 4 + j
                            sl_ = slice(s * DE, (s + 1) * DE)
                            nc.tensor.matmul(pt[:, j * DE:(j + 1) * DE],
                                             lhs_t[:, sl_], rhs_t[:, sl_],
                                             start=True, stop=True)
                        dst = att[(br, half)][:, q * 512:(q + 1) * 512]
                        if br == "v" and half == "hi":
                            nc.scalar.activation(dst, pt[:], ACT.Tanh,
                                                 scale=alph_t[:])
                        else:
                            nc.scalar.activation(dst, pt[:], ACT.Tanh)

                # ---- H = relu(att @ WcT + enc-term) ----
                for q in range(NQ):
                    ck = slice(q * 512, (q + 1) * 512)
                    ph_a = hps.tile([DA, 512], F32, tag="ph_a")
                    nc.tensor.matmul(ph_a[:], waT_t[:], av_a[:, ck],
                                     start=True, stop=False)
                    nc.tensor.matmul(ph_a[:], wca_lo[:],
                                     att[("a", "lo")][:, ck],
                                     start=False, stop=False)
                    nc.tensor.matmul(ph_a[:], wca_hi[:],
                                     att[("a", "hi")][:, ck],
                                     start=False, stop=True)
                    nc.vector.tensor_scalar_max(ht_a[:, ck], ph_a[:], 0.0)
                    ph_v = hps.tile([DA, 512], F32, tag="ph_v")
                    nc.tensor.matmul(ph_v[:], rsWv_t[:], av_v[:, ck],
                                     start=True, stop=False)
                    nc.tensor.matmul(ph_v[:], wcv_lo[:],
                                     att[("v", "lo")][:, ck],
                                     start=False, stop=False)
                    nc.tensor.matmul(ph_v[:], wcv_hi[:],
                                     att[("v", "hi")][:, ck],
                                     start=False, stop=True)
                    nc.vector.tensor_scalar_max(ht_v[:, ck], ph_v[:], 0.0)

                # ---- out[c | 32+c, (s,k)] = Wh @ H + residual ----
                # aud rows 0-2, vis rows 32-34 (32-aligned out positions);
                # stationaries are the tiny Wh/identity matrices (3-col
                # weight loads) instead of 128-col ht/av slices.
                out_sb = sb.tile([35, SK], F32, name="out_sb", tag="out_sb")
                for q in range(NQ):
                    ck = slice(q * 512, (q + 1) * 512)
                    po = ops_.tile([35, 512], F32, tag="po")
                    nc.tensor.matmul(po[0:3, :], wha_t[:], ht_a[:, ck],
                                     start=True, stop=False)
                    nc.tensor.matmul(po[0:3, :], i3_t[:], av_a[:, ck],
                                     start=False, stop=True)
                    nc.tensor.matmul(po[32:35, :], whv_t[:], ht_v[:, ck],
                                     start=True, stop=False)
                    nc.tensor.matmul(po[32:35, :], on13_t[:], av_v[:, ck],
                                     start=False, stop=True)
                    nc.vector.tensor_copy(out_sb[0:3, ck], po[0:3, :])
                    nc.vector.tensor_copy(out_sb[32:35, ck], po[32:35, :])
                nc.sync.dma_start(out[0:3, :], out_sb[0:3, :])
                nc.sync.dma_start(out[3:6, :], out_sb[32:35, :])

    # ---- post-schedule surgery: rewrite the fake-sem wait to xsem ----
    waits = [w for w in fkw.ins.sync_info.on_wait if w.id == fksem.num]
    assert len(waits) == 1, fkw.ins.sync_info.on_wait
    waits[0].id = xsem.num

    all_insts = [i for bb in nc.m.functions[0].blocks for i in bb.instructions]
    sync_eng = [i for i in all_insts if str(i.engine) == "EngineType.SP"]
    spos = {i.name: p for p, i in enumerate(sync_eng)}
    wpos = spos[fkw.ins.name]
    # the wait must precede every partner-block read on the sync stream,
    # and every shared-HBM write must precede the wait (else all cores
    # would block before notifying their partner).
    for d in rd_dmas:
        assert wpos < spos[d.ins.name], \
            f"partner read {d.ins.name} scheduled before xsem wait"
    for w in wr:
        assert spos[w.ins.name] < wpos, \
            f"sh write {w.ins.name} scheduled after xsem wait"
    # ...and the notify trigger (gpsimd) must not depend on post-wait sync
    # instructions (deadlock risk).  The trigger's waits resolve to sem
    # updates from the write DMAs, which we assert are pre-wait.
    post_wait_sync = {i.name for i in sync_eng[wpos:]}
    gpos = {i.name: p for p, i in enumerate(all_insts)}
    sem_updaters = {}
    for i in all_insts:
        si = i.sync_info
        if si is None:
            continue
        for u in si.on_update:
            v = u.update_value if u.update_value else 1
            if v > 0:
                sem_updaters.setdefault(u.id, []).append((gpos[i.name], i, v))
    for ups in sem_updaters.values():
        ups.sort(key=lambda t: t[0])
    seen = set()
    stack = [trig.ins]
    while stack:
        i = stack.pop()
        if i.name in seen:
            continue
        seen.add(i.name)
        assert i.name not in post_wait_sync, \
            f"notify path depends on post-wait sync inst {i.name}"
        si = i.sync_info
        if si is None:
            continue
        for wt in si.on_wait:
            need = wt.wait_value or 0
            acc = 0
            for _, up, v in sem_updaters.get(wt.id, []):
                if acc >= need:
                    break
                acc += v
                if up.name not in seen:
                    stack.append(up)

    nc.compile()
    return nc


_NC_CACHE = None


def _get_nc():
    global _NC_CACHE
    if _NC_CACHE is None:
        _NC_CACHE = build_bass()
    return _NC_CACHE


def _prep_inputs(f1_norm, f2_norm, W1, b1, W2, b2, Aa, Av, Wa, Wv,
                 Wca, Wcv, Wha, Whv):
    f16 = _np_dt(F16)
    f1_norm = np.asarray(f1_norm, dtype=np.float32)
    f2_norm = np.asarray(f2_norm, dtype=np.float32)
    Aa = np.asarray(Aa, dtype=np.float32)
    Av = np.asarray(Av, dtype=np.float32)

    a_ds = f1_norm[:, :, ::4, ::4].reshape(B, 3, D)       # (B, 3, D)
    v_ds = f2_norm[:, ::4, ::4].reshape(B, D)             # (B, D)
    w1T = np.ascontiguousarray(np.asarray(W1).T).astype(f16)   # (D, 128)
    w2T = np.ascontiguousarray(np.asarray(W2).T).astype(f16)

    scale = 1.0 / 16.0
    consts = {
        "bdA": (Aa * scale).astype(f16),
        "rsA": (Aa.sum(axis=1, keepdims=True) * scale).astype(f16),
        "csAv": (Av.sum(axis=0)[:, None] * scale).astype(f16),
        "alph": np.full((128, 1), Av.sum() * scale, np.float32),
        "wcaT": np.ascontiguousarray(np.asarray(Wca).T).astype(f16),
        "wcvT": np.ascontiguousarray(np.asarray(Wcv).T).astype(f16),
        "waT": np.ascontiguousarray(np.asarray(Wa).T).astype(f16),
        "rsWv": np.asarray(Wv).sum(axis=1)[None, :].astype(f16),
        "whaT": np.ascontiguousarray(np.asarray(Wha).T).astype(f16),
        "whvT": np.ascontiguousarray(np.asarray(Whv).T).astype(f16),
        "i3": np.eye(3, dtype=f16),
        "on13": np.ones((1, 3), f16),
    }

    in_maps = []
    for i in range(NC_):
        p, j = divmod(i, 2)
        dsl = slice(j * DL, (j + 1) * DL)
        smp = slice(SP * p, SP * (p + 1))
        a_blk = a_ds[smp][:, :, dsl]                       # (32, 3, 10240)
        v_blk = v_ds[smp][:, dsl]                          # (32, 10240)
        # aT[d, c*32+u] = a_blk[u, c, d];  vT[d, u] = v_blk[u, d]
        aT = np.ascontiguousarray(a_blk.transpose(2, 1, 0)
                                  ).reshape(DL, 96).astype(f16)
        vT = np.ascontiguousarray(v_blk.T).astype(f16)
        pk = np.concatenate([
            aT.reshape(NT, 128, 96), vT.reshape(NT, 128, 32),
            w1T[dsl].reshape(NT, 128, 128), w2T[dsl].reshape(NT, 128, 128),
        ], axis=2).transpose(1, 0, 2).reshape(128, NT * PKW)
        m = {"pk": np.ascontiguousarray(pk)}
        m.update(consts)
        in_maps.append(m)
    return in_maps


def _unshard(res):
    outs = []
    for i in range(NC_):
        arr = res.results[i]["out"].reshape(6, SL, DE)     # [c|3+c, s, k]
        aud = arr[0:3].transpose(1, 0, 2)                  # (16, 3, 128)
        vis = arr[3:6].transpose(1, 0, 2)
        outs.append(np.concatenate([aud, vis], axis=2))    # (16, 3, 256)
    return np.concatenate(outs, axis=0).astype(np.float32, copy=False)


def _run(inputs, trace=False):
    nc = _get_nc()
    in_maps = _prep_inputs(**inputs)
    res = run_bass_kernel_spmd(nc, in_maps, list(range(NC_)), trace=trace)
    return _unshard(res), res


def kernel(**inputs):
    out, _ = _run(inputs, trace=False)
    return out
